# revision 31
# baseline (speedup 1.0000x reference)
"""Multi-head graph attention (GAT) Trainium2 kernel — PE-direct, DoubleRow fp8.

Head-parallel: 8 heads -> 8 NeuronCores, each core computes one head's full
attention over the 4096-node graph.

Math (per head):
    h_prime = h @ w                  [4096, 64]
    s       = h_prime @ a            [4096]
    attn_ij = LeakyReLU_0.2(s_i + s_j), masked by adj_ij, softmax over j
    out     = softmax(attn) @ h_prime + bias, then LeakyReLU_0.01

Key rewrite vs the elementwise baseline: with nodes score-sorted,
exp(LeakyReLU_0.2(s_i+s_j)) = max(u_i u_j, v_i v_j) (u=e^s, v=e^{0.2 s}) is
PIECEWISE RANK-1.  Any per-column factor cancels in the softmax, so columns
can be normalized by 1/v_i, making the masked exp matrix

    E^T[j, i] = adj_ij * ( v_j              for s_i + s_j <  0 (prefix)
                           w_i * u_j        for s_i + s_j >= 0 (suffix)
                           max(w_i u_j, v_j) in the mixed band )   w = e^{0.8 s}

Prefix and suffix need NO elementwise work: the raw 0/1 fp8 adjacency is the
PE's moving operand with host-precomputed fp8 stationaries v_j*hp1 / u_j*hp1
(the latter pre-divided by a global power of two C_B to fit e4m3's +-240
range).  j-tiles are processed in PAIRS with MatmulPerfMode.DoubleRow (2 fp8
MACs/cell/cycle, K=256), halving PE streaming time.  The TOP score pair
(tiles 30-31) instead runs normal-mode bf16, and the band stationary hp1 is
bf16: columns with concentrated attention take most of their mass from these
nodes/elements, where fp8's ~3% error would show up raw in the output.
Only the mixed band (~7% of columns) is built elementwise, per tile.

PSUM holds prefix+suffix accumulators for half the output columns, so the
adjacency streams in two column-half passes (each byte read exactly once,
stored so every 1 MiB two-pair transfer is fully contiguous per partition).
Suffix banks are opened by zero-stationary matmuls that double as PE HAM
warm-up during the initial DMA; in the second half they are deferred until
first use so the PE can restart on prefix work while the previous half's
suffix banks drain.  A 65th ones-column in the stationaries accumulates the
softmax denominator.  The kernel returns the prefix and suffix accumulators
separately as bf16 [65, 4096] tensors (banks drain with plain copies,
alternating ScalarE/VectorE, each issued as soon as its bank closes); the
w_i*C_B suffix scale, combine, divide, bias and output LeakyReLU run on the
host.
"""

import math
import sys

for _p in ("/opt/trn_rl_repo",):
    if _p not in sys.path:
        sys.path.insert(0, _p)

import numpy as np
import ml_dtypes


def _ensure_axon_hooks_stub():
    """bass_utils imports antenv.axon_hooks when BASS_TRACE is set; this image's
    antenv lacks it. Register a no-op stub so tracing degrades gracefully."""
    try:
        from antenv.axon_hooks import get_axon_ntff_profile_hook  # noqa: F401
        return
    except ImportError:
        pass
    import types

    mod = types.ModuleType("antenv.axon_hooks")
    state = {"hook": None}
    mod.set_axon_ntff_profile_hook = lambda h: state.__setitem__("hook", h)
    mod.get_axon_ntff_profile_hook = lambda: state["hook"]
    sys.modules["antenv.axon_hooks"] = mod
    try:
        import antenv

        antenv.axon_hooks = mod
    except ImportError:
        pass


_ensure_axon_hooks_stub()

import concourse.bass as bass
import concourse.tile as tile
from concourse import mybir
from concourse.bass_utils import run_bass_kernel_spmd

BF16 = ml_dtypes.bfloat16
F8 = ml_dtypes.float8_e4m3
N = 4096
F_IN = 256
F_OUT = 64
H = 8
NJT = 32         # j tiles of 128
NPAIR = 16       # DoubleRow j-tile pairs of 256
NPQ = 8          # two pairs per DMA transfer (1 MiB contiguous)
TOPP = NPAIR - 1  # top-score pair handled in bf16 (attention concentrates here)
MPAD = 80        # stationary column pad (DoubleRow needs 16B-aligned k-step)
CHW = 512        # PSUM chunk width (one bank)
HALFW = 2048     # columns per half-pass (4 pre + 4 suf banks)
GR = 16          # column alignment granularity

LAST_RESULTS = None  # BassKernelResults of the most recent run (for test.py)

_CACHED_NC = None
_CACHED_KEY = None


def _cast_bf16(x32: np.ndarray) -> np.ndarray:
    """Fast float32 -> bfloat16 (round-to-nearest-even) via bit twiddling."""
    b = np.ascontiguousarray(x32, dtype=np.float32).view(np.uint32)
    r = (b >> np.uint32(16)) & np.uint32(1)
    out = ((b + np.uint32(0x7FFF) + r) >> np.uint32(16)).astype(np.uint16)
    return out.view(BF16)


def _split_excess_waits(nc: bass.Bass) -> None:
    """Walrus encodes at most one semaphore wait per TPB instruction ("Too
    many sync wait commands"); spill surplus waits onto same-engine NoOps
    placed immediately before the instruction."""
    import bass_rust

    ctr = 0
    for fn in nc.m.functions:
        for blk in fn.blocks:
            out = []
            changed = False
            for inst in blk.instructions:
                limit = 1
                si = inst.sync_info
                if si is not None and len(si.on_wait or []) > limit:
                    waits = list(si.on_wait)
                    spill, keep = waits[:-limit], waits[-limit:]
                    for wsp in spill:
                        ctr += 1
                        out.append(
                            mybir.InstNoOp(
                                name=f"I-waitnop-{ctr}",
                                engine=inst.engine,
                                sync_info=bass_rust.SyncInfo(on_wait=[wsp], on_update=[]),
                            )
                        )
                    inst.sync_info = bass_rust.SyncInfo(
                        on_wait=keep, on_update=list(si.on_update or [])
                    )
                    changed = True
                out.append(inst)
            if changed:
                blk.instructions = out


def plan_half(LOp, HIp, h0, h1):
    """Matmul schedule for one column-half at pair granularity: ordered
    segments with PSUM start/stop flags.  Coverage invariant: a 'zero' fill
    opens the suffix banks and pair 0's prefix+band spans [h0, h1)
    (HIp[0] == N), so every segment is either entirely first-touch or
    entirely accumulate."""
    ncols = (h1 - h0) // GR
    cov = {"pre": bytearray(ncols), "suf": bytearray(ncols)}
    ops = []

    def add(kind, bank, pr, a, b):
        a, b = max(a, h0), min(b, h1)
        if a >= b:
            return
        c0, c1 = (a - h0) // CHW, (b - 1 - h0) // CHW
        for c in range(c0, c1 + 1):
            ca = max(a, h0 + c * CHW)
            cb = min(b, h0 + (c + 1) * CHW)
            seg = cov[bank][(ca - h0) // GR:(cb - h0) // GR]
            vals = set(seg)
            assert len(vals) == 1, f"mixed coverage {kind} pr={pr} [{ca},{cb})"
            start = vals == {0}
            cov[bank][(ca - h0) // GR:(cb - h0) // GR] = b"\x01" * len(seg)
            ops.append(dict(kind=kind, bank=bank, pr=pr, c=c, a=ca, b=cb,
                            start=start, stop=False))

    add("zero", "suf", -1, max(HIp[NPAIR - 1], h0), h1)
    for pr in range(NPAIR):
        add("pre", "pre", pr, h0, min(LOp[pr], h1))
        add("suf", "suf", pr, max(HIp[pr], h0), h1)
        add("band", "pre", pr, max(LOp[pr], h0), min(HIp[pr], h1))
    last = {}
    for i, op in enumerate(ops):
        last[(op["bank"], op["c"])] = i
    for i in last.values():
        ops[i]["stop"] = True
    bypr = {}
    for op in ops:
        bypr.setdefault(op["pr"], []).append(op)
    close = {}  # (bank, chunk) -> pair index whose issue closes the bank
    for op in ops:
        if op["stop"]:
            close[(op["bank"], op["c"])] = op["pr"]
    return bypr, close


def build_nc(splits) -> bass.Bass:
    LOp, HIp = splits
    f32 = mybir.dt.float32
    bf16 = mybir.dt.bfloat16
    f8 = mybir.dt.float8e4
    Alu = mybir.AluOpType
    Act = mybir.ActivationFunctionType
    DR = mybir.MatmulPerfMode.DoubleRow

    nc = bass.Bass()
    # [half, pq, partition, pair-in-transfer, tile-in-pair, column]
    adjm = nc.declare_dram_parameter("adjm", [2, NPQ, 128, 2, 2, HALFW], f8, isOutput=False)
    AstD = nc.declare_dram_parameter("AstD", [128, NPAIR, 2, MPAD], f8, isOutput=False)
    BstD = nc.declare_dram_parameter("BstD", [128, NPAIR, 2, MPAD], f8, isOutput=False)
    AstT = nc.declare_dram_parameter("AstT", [128, 2, F_OUT + 1], bf16, isOutput=False)
    BstT = nc.declare_dram_parameter("BstT", [128, 2, F_OUT + 1], bf16, isOutput=False)
    hp1D = nc.declare_dram_parameter("hp1D", [128, NJT, F_OUT + 1], bf16, isOutput=False)
    uv = nc.declare_dram_parameter("uv", [128, 2, NJT], f32, isOutput=False)
    wrowh = nc.declare_dram_parameter("wrowh", [N], bf16, isOutput=False)
    outP_d = nc.declare_dram_parameter("outP", [F_OUT + 1, N], bf16, isOutput=True)
    outS_d = nc.declare_dram_parameter("outS", [F_OUT + 1, N], bf16, isOutput=True)

    dma_engines = [nc.sync, nc.scalar, nc.gpsimd]

    with tile.TileContext(nc) as tc:
        with tc.tile_pool(name="persist", bufs=1) as persist:
            uv_sb = persist.tile([128, 2, NJT], f32)          # u | v scalar columns
            wrow = persist.tile([128, N], bf16)               # e^{0.8 s_i} bcast down parts
            zcol = persist.tile([1, F_OUT + 1], bf16)         # zero stationary (psum opener)
            zrow = persist.tile([1, CHW], bf16)               # zero moving row for fills
            Ast = persist.tile([128, NPAIR, 2, MPAD], f8)     # v_j * hp1 (prefix, paired)
            Bst = persist.tile([128, NPAIR, 2, MPAD], f8)     # u_j * hp1 / CB (suffix, paired)
            AstTs = persist.tile([128, 2, F_OUT + 1], bf16)   # top-pair bf16 stationaries
            BstTs = persist.tile([128, 2, F_OUT + 1], bf16)
            hp1t = persist.tile([128, NJT, F_OUT + 1], bf16)  # raw hp1 (band stationary)

            nc.vector.memset(zcol[:], 0.0)
            nc.vector.memset(zrow[:], 0.0)
            with tc.high_priority():
                nc.sync.dma_start(out=Ast[:], in_=AstD[:])
                nc.scalar.dma_start(out=Bst[:], in_=BstD[:])
                nc.scalar.dma_start(out=uv_sb[:], in_=uv[:])
                nc.gpsimd.dma_start(out=AstTs[:], in_=AstT[:])
                nc.gpsimd.dma_start(out=BstTs[:], in_=BstT[:])

            with (
                tc.tile_pool(name="adj_stream", bufs=8) as ap_,
                tc.tile_pool(name="band", bufs=4) as bp,
                tc.tile_pool(name="psum_acc", bufs=8, space="PSUM") as pacc,
                tc.tile_pool(name="fin", bufs=1) as fin,
            ):
                oP = fin.tile([F_OUT + 1, N], bf16)   # prefix+band accumulator
                oS = fin.tile([F_OUT + 1, N], bf16)   # suffix accumulator (unscaled)
                drain_rr = [0]  # alternates the drain copy between ScalarE/VectorE

                for half in range(2):
                    h0, h1 = half * HALFW, (half + 1) * HALFW
                    accP = [
                        pacc.tile([F_OUT + 1, CHW], f32, tag="acc", name=f"accP_{half}_{c}")
                        for c in range(4)
                    ]
                    accS = [
                        pacc.tile([F_OUT + 1, CHW], f32, tag="acc", name=f"accS_{half}_{c}")
                        for c in range(4)
                    ]

                    def bank(op):
                        return (accP if op["bank"] == "pre" else accS)[op["c"]]

                    bypr, close = plan_half(LOp, HIp, h0, h1)
                    closers = {}
                    for (bk, c), pr in close.items():
                        closers.setdefault(pr, []).append((bk, c))

                    def issue(op, rhs_ap, lhs, perf_mode=None, start=None, stop=None):
                        t = bank(op)
                        c0 = h0 + op["c"] * CHW
                        nc.tensor.matmul(
                            t[:, op["a"] - c0:op["b"] - c0], lhs, rhs_ap,
                            start=op["start"] if start is None else start,
                            stop=op["stop"] if stop is None else stop,
                            perf_mode=perf_mode,
                        )

                    def drain(bk, c):
                        c0 = h0 + c * CHW
                        sl = slice(c0, c0 + CHW)
                        dst, src = (oP, accP[c]) if bk == "pre" else (oS, accS[c])
                        if drain_rr[0] % 2 == 0:
                            nc.scalar.activation(dst[:, sl], src[:], Act.Copy)
                        else:
                            nc.vector.tensor_copy(dst[:, sl], src[:])
                        drain_rr[0] += 1

                    def issue_zeros(rounds=1):
                        for r in range(rounds):
                            for op in bypr.get(-1, []):
                                issue(op, zrow[0:1, 0:op["b"] - op["a"]], zcol[:],
                                      start=op["start"] if r == 0 else False,
                                      stop=False)
                        for bk, c in closers.get(-1, []):  # bank closed by fill alone
                            drain(bk, c)

                    # half 0: suffix-bank fills run first and double as HAM
                    # warm-up while the first adjacency transfer is in flight.
                    # half 1: defer so the PE restarts on prefix matmuls
                    # without waiting for the previous half's suffix drains.
                    zeros_pending = True
                    if half == 0:
                        issue_zeros(rounds=3)
                        zeros_pending = False

                    for pq in range(NPQ):
                        ab8 = ap_.tile([128, 2, 2, HALFW], f8, tag="adjm")
                        # adjacency rides the two HWDGE queues only: the
                        # in-order consumer would head-of-line block on the
                        # slower SWDGE (gpsimd) queue
                        eng = dma_engines[(half * NPQ + pq) % 2]
                        if half == 0 and pq == 0:
                            # first transfer split per pair across both HWDGE
                            # queues: pair 0's matmuls start half a transfer
                            # sooner
                            for q in range(2):
                                with tc.high_priority():
                                    dma_engines[q].dma_start(
                                        out=ab8[:, q], in_=adjm[half][pq][:, q]
                                    )
                        elif half == 0 and pq == 1:
                            with tc.high_priority():
                                eng.dma_start(out=ab8[:], in_=adjm[half][pq])
                        else:
                            eng.dma_start(out=ab8[:], in_=adjm[half][pq])
                        if half == 0 and pq == 0:
                            # needed only from the first banded pair (~7) on;
                            # gpsimd is otherwise idle for DMA
                            nc.gpsimd.dma_start(out=hp1t[:], in_=hp1D[:])
                            nc.gpsimd.dma_start(
                                out=wrow[:, 0:N // 2],
                                in_=wrowh[0:N // 2].partition_broadcast(128),
                            )
                            nc.gpsimd.dma_start(
                                out=wrow[:, N // 2:N],
                                in_=wrowh[N // 2:N].partition_broadcast(128),
                            )
                        for q in range(2):
                            pr = pq * 2 + q
                            abq = ab8[:, q]
                            pops = bypr.get(pr, [])
                            # fp8 DoubleRow regions first (PE never waits on band work)
                            for op in pops:
                                if op["kind"] not in ("pre", "suf"):
                                    continue
                                if op["kind"] == "suf" and zeros_pending:
                                    issue_zeros()
                                    zeros_pending = False
                                if pr == TOPP:  # bf16 per-tile matmuls, top pair
                                    lhs_t = AstTs if op["kind"] == "pre" else BstTs
                                    for e in range(2):
                                        issue(op, abq[:, e, op["a"] - h0:op["b"] - h0],
                                              lhs_t[:, e, :],
                                              start=op["start"] and e == 0,
                                              stop=op["stop"] and e == 1)
                                else:
                                    lhs_p = Ast if op["kind"] == "pre" else Bst
                                    issue(op, abq[:, :, op["a"] - h0:op["b"] - h0],
                                          lhs_p[:, pr, :, 0:F_OUT + 1], perf_mode=DR)
                            # suffix banks see their last write here; drain
                            # before the band work so the copies overlap it
                            for bk, c in closers.get(pr, []):
                                if bk == "suf":
                                    drain(bk, c)
                            l = max(LOp[pr], h0)
                            h = min(HIp[pr], h1)
                            if l < h:
                                w = h - l
                                ets = []
                                for e in range(2):
                                    jt = 2 * pr + e
                                    u_j = uv_sb[:, 0, jt:jt + 1]
                                    v_j = uv_sb[:, 1, jt:jt + 1]
                                    abb = bp.tile([128, CHW], bf16, tag=f"abb{e}")
                                    kt = bp.tile([128, CHW], bf16, tag=f"kt{e}")
                                    et = bp.tile([128, CHW], bf16, tag=f"et{e}")
                                    nc.scalar.activation(
                                        abb[:, 0:w], abq[:, e, l - h0:h - h0], Act.Copy
                                    )
                                    nc.vector.tensor_scalar(
                                        kt[:, 0:w], wrow[:, l:h], u_j, v_j,
                                        op0=Alu.mult, op1=Alu.max,
                                    )
                                    nc.vector.tensor_mul(et[:, 0:w], kt[:, 0:w], abb[:, 0:w])
                                    ets.append(et)
                                for op in pops:
                                    if op["kind"] == "band":
                                        for e in range(2):
                                            jt = 2 * pr + e
                                            issue(
                                                op, ets[e][:, op["a"] - l:op["b"] - l],
                                                hp1t[:, jt, :],
                                                start=op["start"] and e == 0,
                                                stop=op["stop"] and e == 1,
                                            )
                            # drain any pre banks whose accumulation closed here
                            for bk, c in closers.get(pr, []):
                                if bk == "pre":
                                    drain(bk, c)

                    # store this half (bank copies above already freed PSUM)
                    hsl = slice(h0, h1)
                    nc.gpsimd.dma_start(out=outP_d[:, hsl], in_=oP[:, hsl])
                    nc.gpsimd.dma_start(out=outS_d[:, hsl], in_=oS[:, hsl])
    return nc


def kernel(h, adj, w, a_src, bias, **_unused):
    global LAST_RESULTS, _CACHED_NC, _CACHED_KEY
    h = np.asarray(h, dtype=np.float32)
    adj = np.asarray(adj)
    w = np.asarray(w, dtype=np.float32)
    a_src = np.asarray(a_src, dtype=np.float32)
    bias = np.asarray(bias, dtype=np.float32)

    adj_u8 = adj.astype(np.uint8)

    # Per-head score-sorted node permutation: makes the sign of s_i + s_j
    # constant per (j-pair, column-range) so prefix/suffix regions are
    # contiguous column spans shared (via min/max) across heads.
    perms, s_sorted_all = [], []
    for c in range(H):
        s_host = (
            h.astype(np.float64)
            @ (w[c].astype(np.float64) @ a_src[c].astype(np.float64))[:, 0]
        )
        perm = np.argsort(s_host, kind="stable")
        perms.append(perm)
        s_sorted_all.append(s_host[perm])

    lo_all = np.array(
        [np.searchsorted(ss, -ss[255::256]) for ss in s_sorted_all]
    )  # [H, NPAIR]
    hi_all = np.array(
        [np.searchsorted(ss, -ss[0::256]) for ss in s_sorted_all]
    )
    LOp = np.clip(lo_all.min(axis=0) // GR * GR, 0, N)
    HIp = np.clip(-(-hi_all.max(axis=0) // GR) * GR, 0, N)
    HIp = np.maximum(HIp, LOp)
    HIp[0] = N  # pair 0's prefix+band must span all columns (psum start flags)
    LOp, HIp = [int(x) for x in LOp], [int(x) for x in HIp]
    SUF0 = HIp[NPAIR - 1]
    assert max(hh - ll for ll, hh in zip(LOp, HIp)) <= CHW, "mixed band exceeds et tile"

    one_f8 = np.array(1.0, dtype=F8).view(np.uint8)

    def to_pair_stat(x65):  # [4096, 65] f32 -> [128, NPAIR, 2, MPAD] fp8
        t = np.zeros((128, NPAIR, 2, MPAD), np.float32)
        t[:, :, :, 0:F_OUT + 1] = x65.reshape(NPAIR, 2, 128, F_OUT + 1).transpose(2, 0, 1, 3)
        assert np.abs(t).max() <= 240.0, "fp8 e4m3 overflow in stationary"
        return np.ascontiguousarray(t.astype(F8))

    def to_top_stat(x65):  # top-pair rows [3840:4096] -> [128, 2, 65] bf16
        return _cast_bf16(np.ascontiguousarray(
            x65[-256:].reshape(2, 128, F_OUT + 1).transpose(1, 0, 2)
        ))

    # global power-of-two scale so u_j*hp1 fits e4m3 (max 240); applied back
    # on the host during the suffix combine.  Shared across heads (SPMD).
    maxB = 0.0
    hps, us, vs = [], [], []
    for c in range(H):
        perm, ss = perms[c], s_sorted_all[c]
        hp = (h[perm].astype(np.float64) @ w[c].astype(np.float64)).astype(np.float32)
        hp1 = np.concatenate([hp, np.ones((N, 1), np.float32)], axis=1)
        u_full = np.exp(ss).astype(np.float32)
        v_full = np.exp(0.2 * ss).astype(np.float32)
        maxB = max(maxB, float(np.abs(hp1[:-256] * u_full[:-256, None]).max()))
        hps.append(hp1); us.append(u_full); vs.append(v_full)
    CB = 2 ** math.ceil(math.log2(maxB / 240.0))

    in_maps = []
    for c in range(H):
        perm, ss = perms[c], s_sorted_all[c]
        # paired blocked permuted transposed adjacency, half-major, two pairs
        # per contiguous transfer block:
        # adjm[half, pq, p, q, e, i'] = adj[perm[half*2048+i'], perm[((2pq+q)*2+e)*128+p]]
        G = adj_u8[perm][:, perm]
        blk_p = (np.ascontiguousarray(G.T).reshape(NPQ, 2, 2, 128, N) * one_f8)
        adjm_c = np.ascontiguousarray(
            blk_p.reshape(NPQ, 2, 2, 128, 2, HALFW).transpose(4, 0, 3, 1, 2, 5)
        ).view(F8)

        hp1, u_full, v_full = hps[c], us[c], vs[c]
        Bfull = hp1 * u_full[:, None] / CB
        Bfull[-256:] = 0.0  # top pair runs the bf16 path; keep fp8 in range
        s_col = ss.reshape(NJT, 128).T
        uv_c = np.stack(
            [np.exp(s_col), np.exp(0.2 * s_col)], axis=1
        ).astype(np.float32)
        wrow_c = _cast_bf16(np.exp(0.8 * ss).astype(np.float32))
        hp1_bf = _cast_bf16(np.ascontiguousarray(
            hp1.reshape(NJT, 128, F_OUT + 1).transpose(1, 0, 2)
        ))
        in_maps.append(
            {
                "adjm": adjm_c,
                "AstD": to_pair_stat(hp1 * v_full[:, None]),
                "BstD": to_pair_stat(Bfull),
                "AstT": to_top_stat(hp1 * v_full[:, None]),
                "BstT": to_top_stat(hp1 * u_full[:, None] / CB),
                "hp1D": hp1_bf,
                "uv": np.ascontiguousarray(uv_c),
                "wrowh": wrow_c,
            }
        )

    key = (tuple(LOp), tuple(HIp))
    if _CACHED_NC is None or _CACHED_KEY != key:
        _CACHED_NC = build_nc((LOp, HIp))
        _split_excess_waits(_CACHED_NC)  # HW-only fixup; CoreSim rejects the NoOps
        _CACHED_KEY = key
    res = run_bass_kernel_spmd(_CACHED_NC, in_maps, list(range(H)))
    LAST_RESULTS = res

    # host finalize: combine prefix + CB*w_i*suffix, divide by rowsum, bias,
    # output LeakyReLU(0.01), unpermute
    out = np.empty((H, N, F_OUT), dtype=np.float32)
    for c in range(H):
        P = np.asarray(res.results[c]["outP"]).astype(np.float64)  # [65, 4096]
        S = np.asarray(res.results[c]["outS"]).astype(np.float64)
        S[:, :SUF0] = 0.0
        wr = np.exp(0.8 * s_sorted_all[c]) * CB
        t = P + S * wr[None, :]
        a = (t[0:F_OUT] / t[F_OUT:F_OUT + 1]).T + bias[None, :]
        out[c, perms[c], :] = np.where(a >= 0, a, 0.01 * a)
    return out


# revision 32
# speedup vs baseline: 1.0479x; 1.0479x over previous
"""Multi-head graph attention (GAT) Trainium2 kernel — PE-direct, DoubleRow fp8.

Head-parallel: 8 heads -> 8 NeuronCores, each core computes one head's full
attention over the 4096-node graph.

Math (per head):
    h_prime = h @ w                  [4096, 64]
    s       = h_prime @ a            [4096]
    attn_ij = LeakyReLU_0.2(s_i + s_j), masked by adj_ij, softmax over j
    out     = softmax(attn) @ h_prime + bias, then LeakyReLU_0.01

Key rewrite vs the elementwise baseline: with nodes score-sorted,
exp(LeakyReLU_0.2(s_i+s_j)) = max(u_i u_j, v_i v_j) (u=e^s, v=e^{0.2 s}) is
PIECEWISE RANK-1.  Any per-column factor cancels in the softmax, so columns
can be normalized by 1/v_i, making the masked exp matrix

    E^T[j, i] = adj_ij * ( v_j              for s_i + s_j <  0 (prefix)
                           w_i * u_j        for s_i + s_j >= 0 (suffix)
                           max(w_i u_j, v_j) in the mixed band )   w = e^{0.8 s}

Prefix and suffix need NO elementwise work: the raw 0/1 fp8 adjacency is the
PE's moving operand with host-precomputed fp8 stationaries v_j*hp1 / u_j*hp1
(the latter pre-divided by a global power of two C_B to fit e4m3's +-240
range).  j-tiles are processed in PAIRS with MatmulPerfMode.DoubleRow (2 fp8
MACs/cell/cycle, K=256), halving PE streaming time.  The TOP score pair
(tiles 30-31) instead runs normal-mode bf16, and the band stationary hp1 is
bf16: columns with concentrated attention take most of their mass from these
nodes/elements, where fp8's ~3% error would show up raw in the output.
Only the mixed band (~7% of columns) is built elementwise, per tile.

PSUM holds prefix+suffix accumulators for half the output columns, so the
adjacency streams in two column-half passes (each byte read exactly once,
stored so every 1 MiB two-pair transfer is fully contiguous per partition).
Suffix banks are opened by zero-stationary matmuls that double as PE HAM
warm-up during the initial DMA; in the second half they are deferred until
first use so the PE can restart on prefix work while the previous half's
suffix banks drain.  A 65th ones-column in the stationaries accumulates the
softmax denominator.  The kernel returns the prefix and suffix accumulators
separately as bf16 [65, 4096] tensors (banks drain with plain copies,
alternating ScalarE/VectorE, each issued as soon as its bank closes); the
w_i*C_B suffix scale, combine, divide, bias and output LeakyReLU run on the
host.
"""

import math
import sys

for _p in ("/opt/trn_rl_repo",):
    if _p not in sys.path:
        sys.path.insert(0, _p)

import numpy as np
import ml_dtypes


def _ensure_axon_hooks_stub():
    """bass_utils imports antenv.axon_hooks when BASS_TRACE is set; this image's
    antenv lacks it. Register a no-op stub so tracing degrades gracefully."""
    try:
        from antenv.axon_hooks import get_axon_ntff_profile_hook  # noqa: F401
        return
    except ImportError:
        pass
    import types

    mod = types.ModuleType("antenv.axon_hooks")
    state = {"hook": None}
    mod.set_axon_ntff_profile_hook = lambda h: state.__setitem__("hook", h)
    mod.get_axon_ntff_profile_hook = lambda: state["hook"]
    sys.modules["antenv.axon_hooks"] = mod
    try:
        import antenv

        antenv.axon_hooks = mod
    except ImportError:
        pass


_ensure_axon_hooks_stub()

import concourse.bass as bass
import concourse.tile as tile
from concourse import mybir
from concourse.bass_utils import run_bass_kernel_spmd

BF16 = ml_dtypes.bfloat16
F8 = ml_dtypes.float8_e4m3
N = 4096
F_IN = 256
F_OUT = 64
H = 8
NJT = 32         # j tiles of 128
NPAIR = 16       # DoubleRow j-tile pairs of 256
NPQ = 8          # two pairs per DMA transfer (1 MiB contiguous)
TOPP = NPAIR - 1  # top-score pair handled in bf16 (attention concentrates here)
MPAD = 80        # stationary column pad (DoubleRow needs 16B-aligned k-step)
CHW = 512        # PSUM chunk width (one bank)
HALFW = 2048     # columns per half-pass (4 pre + 4 suf banks)
GR = 16          # column alignment granularity

LAST_RESULTS = None  # BassKernelResults of the most recent run (for test.py)

_CACHED_NC = None
_CACHED_KEY = None


def _cast_bf16(x32: np.ndarray) -> np.ndarray:
    """Fast float32 -> bfloat16 (round-to-nearest-even) via bit twiddling."""
    b = np.ascontiguousarray(x32, dtype=np.float32).view(np.uint32)
    r = (b >> np.uint32(16)) & np.uint32(1)
    out = ((b + np.uint32(0x7FFF) + r) >> np.uint32(16)).astype(np.uint16)
    return out.view(BF16)


def _split_excess_waits(nc: bass.Bass) -> None:
    """Walrus encodes at most one semaphore wait per TPB instruction ("Too
    many sync wait commands"); spill surplus waits onto same-engine NoOps
    placed immediately before the instruction."""
    import bass_rust

    ctr = 0
    for fn in nc.m.functions:
        for blk in fn.blocks:
            out = []
            changed = False
            for inst in blk.instructions:
                limit = 1
                si = inst.sync_info
                if si is not None and len(si.on_wait or []) > limit:
                    waits = list(si.on_wait)
                    spill, keep = waits[:-limit], waits[-limit:]
                    for wsp in spill:
                        ctr += 1
                        out.append(
                            mybir.InstNoOp(
                                name=f"I-waitnop-{ctr}",
                                engine=inst.engine,
                                sync_info=bass_rust.SyncInfo(on_wait=[wsp], on_update=[]),
                            )
                        )
                    inst.sync_info = bass_rust.SyncInfo(
                        on_wait=keep, on_update=list(si.on_update or [])
                    )
                    changed = True
                out.append(inst)
            if changed:
                blk.instructions = out


def plan_half(LOp, HIp, h0, h1):
    """Matmul schedule for one column-half at pair granularity: ordered
    segments with PSUM start/stop flags.  Coverage invariant: a 'zero' fill
    opens the suffix banks and pair 0's prefix+band spans [h0, h1)
    (HIp[0] == N), so every segment is either entirely first-touch or
    entirely accumulate."""
    ncols = (h1 - h0) // GR
    cov = {"pre": bytearray(ncols), "suf": bytearray(ncols)}
    ops = []

    def add(kind, bank, pr, a, b):
        a, b = max(a, h0), min(b, h1)
        if a >= b:
            return
        c0, c1 = (a - h0) // CHW, (b - 1 - h0) // CHW
        for c in range(c0, c1 + 1):
            ca = max(a, h0 + c * CHW)
            cb = min(b, h0 + (c + 1) * CHW)
            seg = cov[bank][(ca - h0) // GR:(cb - h0) // GR]
            vals = set(seg)
            assert len(vals) == 1, f"mixed coverage {kind} pr={pr} [{ca},{cb})"
            start = vals == {0}
            cov[bank][(ca - h0) // GR:(cb - h0) // GR] = b"\x01" * len(seg)
            ops.append(dict(kind=kind, bank=bank, pr=pr, c=c, a=ca, b=cb,
                            start=start, stop=False))

    add("zero", "suf", -1, max(HIp[NPAIR - 1], h0), h1)
    for pr in range(NPAIR):
        add("pre", "pre", pr, h0, min(LOp[pr], h1))
        add("suf", "suf", pr, max(HIp[pr], h0), h1)
        add("band", "pre", pr, max(LOp[pr], h0), min(HIp[pr], h1))
    last = {}
    for i, op in enumerate(ops):
        last[(op["bank"], op["c"])] = i
    for i in last.values():
        ops[i]["stop"] = True
    bypr = {}
    for op in ops:
        bypr.setdefault(op["pr"], []).append(op)
    close = {}  # (bank, chunk) -> pair index whose issue closes the bank
    for op in ops:
        if op["stop"]:
            close[(op["bank"], op["c"])] = op["pr"]
    return bypr, close


def build_nc(splits) -> bass.Bass:
    LOp, HIp = splits
    f32 = mybir.dt.float32
    bf16 = mybir.dt.bfloat16
    f8 = mybir.dt.float8e4
    Alu = mybir.AluOpType
    Act = mybir.ActivationFunctionType
    DR = mybir.MatmulPerfMode.DoubleRow

    nc = bass.Bass()
    # [half, pq, partition, pair-in-transfer, tile-in-pair, column]
    adjm = nc.declare_dram_parameter("adjm", [2, NPQ, 128, 2, 2, HALFW], f8, isOutput=False)
    AstD = nc.declare_dram_parameter("AstD", [128, NPAIR, 2, MPAD], f8, isOutput=False)
    BstD = nc.declare_dram_parameter("BstD", [128, NPAIR, 2, MPAD], f8, isOutput=False)
    AstT = nc.declare_dram_parameter("AstT", [128, 2, F_OUT + 1], bf16, isOutput=False)
    BstT = nc.declare_dram_parameter("BstT", [128, 2, F_OUT + 1], bf16, isOutput=False)
    hp1D = nc.declare_dram_parameter("hp1D", [128, NJT, F_OUT + 1], bf16, isOutput=False)
    uv = nc.declare_dram_parameter("uv", [128, 2, NJT], f32, isOutput=False)
    wrowh = nc.declare_dram_parameter("wrowh", [N], bf16, isOutput=False)
    outP_d = nc.declare_dram_parameter("outP", [F_OUT + 1, N], bf16, isOutput=True)
    outS_d = nc.declare_dram_parameter("outS", [F_OUT + 1, N], bf16, isOutput=True)

    dma_engines = [nc.sync, nc.scalar, nc.gpsimd]

    with tile.TileContext(nc) as tc:
        with tc.tile_pool(name="persist", bufs=1) as persist:
            uv_sb = persist.tile([128, 2, NJT], f32)          # u | v scalar columns
            wrow = persist.tile([128, N], bf16)               # e^{0.8 s_i} bcast down parts
            zcol = persist.tile([1, F_OUT + 1], bf16)         # zero stationary (psum opener)
            zrow = persist.tile([1, CHW], bf16)               # zero moving row for fills
            Ast = persist.tile([128, NPAIR, 2, MPAD], f8)     # v_j * hp1 (prefix, paired)
            Bst = persist.tile([128, NPAIR, 2, MPAD], f8)     # u_j * hp1 / CB (suffix, paired)
            AstTs = persist.tile([128, 2, F_OUT + 1], bf16)   # top-pair bf16 stationaries
            BstTs = persist.tile([128, 2, F_OUT + 1], bf16)
            hp1t = persist.tile([128, NJT, F_OUT + 1], bf16)  # raw hp1 (band stationary)

            nc.vector.memset(zcol[:], 0.0)
            nc.vector.memset(zrow[:], 0.0)
            with tc.high_priority():
                nc.sync.dma_start(out=Ast[:], in_=AstD[:])
                nc.scalar.dma_start(out=Bst[:], in_=BstD[:])
                nc.scalar.dma_start(out=uv_sb[:], in_=uv[:])
                nc.gpsimd.dma_start(out=AstTs[:], in_=AstT[:])
                nc.gpsimd.dma_start(out=BstTs[:], in_=BstT[:])

            with (
                tc.tile_pool(name="adj_stream", bufs=8) as ap_,
                tc.tile_pool(name="band", bufs=4) as bp,
                tc.tile_pool(name="psum_acc", bufs=8, space="PSUM") as pacc,
                tc.tile_pool(name="fin", bufs=1) as fin,
            ):
                oP = fin.tile([F_OUT + 1, N], bf16)   # prefix+band accumulator
                oS = fin.tile([F_OUT + 1, N], bf16)   # suffix accumulator (unscaled)
                drain_rr = [0]  # alternates the drain copy between ScalarE/VectorE

                for half in range(2):
                    h0, h1 = half * HALFW, (half + 1) * HALFW
                    accP = [
                        pacc.tile([F_OUT + 1, CHW], f32, tag="acc", name=f"accP_{half}_{c}")
                        for c in range(4)
                    ]
                    accS = [
                        pacc.tile([F_OUT + 1, CHW], f32, tag="acc", name=f"accS_{half}_{c}")
                        for c in range(4)
                    ]

                    def bank(op):
                        return (accP if op["bank"] == "pre" else accS)[op["c"]]

                    bypr, close = plan_half(LOp, HIp, h0, h1)
                    closers = {}
                    for (bk, c), pr in close.items():
                        closers.setdefault(pr, []).append((bk, c))

                    def issue(op, rhs_ap, lhs, perf_mode=None, start=None, stop=None):
                        t = bank(op)
                        c0 = h0 + op["c"] * CHW
                        nc.tensor.matmul(
                            t[:, op["a"] - c0:op["b"] - c0], lhs, rhs_ap,
                            start=op["start"] if start is None else start,
                            stop=op["stop"] if stop is None else stop,
                            perf_mode=perf_mode,
                        )

                    def drain(bk, c):
                        c0 = h0 + c * CHW
                        sl = slice(c0, c0 + CHW)
                        dst, src = (oP, accP[c]) if bk == "pre" else (oS, accS[c])
                        if drain_rr[0] % 2 == 0:
                            nc.scalar.activation(dst[:, sl], src[:], Act.Copy)
                        else:
                            nc.vector.tensor_copy(dst[:, sl], src[:])
                        drain_rr[0] += 1

                    def issue_zeros(rounds=1):
                        for r in range(rounds):
                            for op in bypr.get(-1, []):
                                issue(op, zrow[0:1, 0:op["b"] - op["a"]], zcol[:],
                                      start=op["start"] if r == 0 else False,
                                      stop=False)
                        for bk, c in closers.get(-1, []):  # bank closed by fill alone
                            drain(bk, c)

                    # half 0: suffix-bank fills run first and double as HAM
                    # warm-up while the first adjacency transfer is in flight.
                    # half 1: defer so the PE restarts on prefix matmuls
                    # without waiting for the previous half's suffix drains.
                    zeros_pending = True
                    if half == 0:
                        issue_zeros(rounds=4)
                        zeros_pending = False

                    for pq in range(NPQ):
                        ab8 = ap_.tile([128, 2, 2, HALFW], f8, tag="adjm")
                        # adjacency rides the two HWDGE queues only: the
                        # in-order consumer would head-of-line block on the
                        # slower SWDGE (gpsimd) queue
                        eng = dma_engines[(half * NPQ + pq) % 2]
                        if half == 0 and pq == 0:
                            # first transfer split per pair across both HWDGE
                            # queues: pair 0's matmuls start half a transfer
                            # sooner
                            for q in range(2):
                                with tc.high_priority():
                                    dma_engines[q].dma_start(
                                        out=ab8[:, q], in_=adjm[half][pq][:, q]
                                    )
                        elif half == 0 and pq == 1:
                            with tc.high_priority():
                                eng.dma_start(out=ab8[:], in_=adjm[half][pq])
                        else:
                            eng.dma_start(out=ab8[:], in_=adjm[half][pq])
                        if half == 0 and pq == 0:
                            # needed only from the first banded pair (~7) on;
                            # gpsimd is otherwise idle for DMA
                            nc.gpsimd.dma_start(out=hp1t[:], in_=hp1D[:])
                            nc.gpsimd.dma_start(
                                out=wrow[:, 0:N // 2],
                                in_=wrowh[0:N // 2].partition_broadcast(128),
                            )
                            nc.gpsimd.dma_start(
                                out=wrow[:, N // 2:N],
                                in_=wrowh[N // 2:N].partition_broadcast(128),
                            )
                        for q in range(2):
                            pr = pq * 2 + q
                            abq = ab8[:, q]
                            pops = bypr.get(pr, [])
                            # fp8 DoubleRow regions first (PE never waits on band work)
                            for op in pops:
                                if op["kind"] not in ("pre", "suf"):
                                    continue
                                if op["kind"] == "suf" and zeros_pending:
                                    issue_zeros()
                                    zeros_pending = False
                                if pr == TOPP:  # bf16 per-tile matmuls, top pair
                                    lhs_t = AstTs if op["kind"] == "pre" else BstTs
                                    for e in range(2):
                                        issue(op, abq[:, e, op["a"] - h0:op["b"] - h0],
                                              lhs_t[:, e, :],
                                              start=op["start"] and e == 0,
                                              stop=op["stop"] and e == 1)
                                else:
                                    lhs_p = Ast if op["kind"] == "pre" else Bst
                                    issue(op, abq[:, :, op["a"] - h0:op["b"] - h0],
                                          lhs_p[:, pr, :, 0:F_OUT + 1], perf_mode=DR)
                            # suffix banks see their last write here; drain
                            # before the band work so the copies overlap it
                            for bk, c in closers.get(pr, []):
                                if bk == "suf":
                                    drain(bk, c)
                            l = max(LOp[pr], h0)
                            h = min(HIp[pr], h1)
                            if l < h:
                                w = h - l
                                ets = []
                                for e in range(2):
                                    jt = 2 * pr + e
                                    u_j = uv_sb[:, 0, jt:jt + 1]
                                    v_j = uv_sb[:, 1, jt:jt + 1]
                                    abb = bp.tile([128, CHW], bf16, tag=f"abb{e}")
                                    kt = bp.tile([128, CHW], bf16, tag=f"kt{e}")
                                    et = bp.tile([128, CHW], bf16, tag=f"et{e}")
                                    nc.scalar.activation(
                                        abb[:, 0:w], abq[:, e, l - h0:h - h0], Act.Copy
                                    )
                                    nc.vector.tensor_scalar(
                                        kt[:, 0:w], wrow[:, l:h], u_j, v_j,
                                        op0=Alu.mult, op1=Alu.max,
                                    )
                                    nc.vector.tensor_mul(et[:, 0:w], kt[:, 0:w], abb[:, 0:w])
                                    ets.append(et)
                                for op in pops:
                                    if op["kind"] == "band":
                                        for e in range(2):
                                            jt = 2 * pr + e
                                            issue(
                                                op, ets[e][:, op["a"] - l:op["b"] - l],
                                                hp1t[:, jt, :],
                                                start=op["start"] and e == 0,
                                                stop=op["stop"] and e == 1,
                                            )
                            # drain any pre banks whose accumulation closed here
                            for bk, c in closers.get(pr, []):
                                if bk == "pre":
                                    drain(bk, c)

                    # store this half (bank copies above already freed PSUM)
                    hsl = slice(h0, h1)
                    nc.gpsimd.dma_start(out=outP_d[:, hsl], in_=oP[:, hsl])
                    nc.gpsimd.dma_start(out=outS_d[:, hsl], in_=oS[:, hsl])
    return nc


def kernel(h, adj, w, a_src, bias, **_unused):
    global LAST_RESULTS, _CACHED_NC, _CACHED_KEY
    h = np.asarray(h, dtype=np.float32)
    adj = np.asarray(adj)
    w = np.asarray(w, dtype=np.float32)
    a_src = np.asarray(a_src, dtype=np.float32)
    bias = np.asarray(bias, dtype=np.float32)

    adj_u8 = adj.astype(np.uint8)

    # Per-head score-sorted node permutation: makes the sign of s_i + s_j
    # constant per (j-pair, column-range) so prefix/suffix regions are
    # contiguous column spans shared (via min/max) across heads.
    perms, s_sorted_all = [], []
    for c in range(H):
        s_host = (
            h.astype(np.float64)
            @ (w[c].astype(np.float64) @ a_src[c].astype(np.float64))[:, 0]
        )
        perm = np.argsort(s_host, kind="stable")
        perms.append(perm)
        s_sorted_all.append(s_host[perm])

    lo_all = np.array(
        [np.searchsorted(ss, -ss[255::256]) for ss in s_sorted_all]
    )  # [H, NPAIR]
    hi_all = np.array(
        [np.searchsorted(ss, -ss[0::256]) for ss in s_sorted_all]
    )
    LOp = np.clip(lo_all.min(axis=0) // GR * GR, 0, N)
    HIp = np.clip(-(-hi_all.max(axis=0) // GR) * GR, 0, N)
    HIp = np.maximum(HIp, LOp)
    HIp[0] = N  # pair 0's prefix+band must span all columns (psum start flags)
    LOp, HIp = [int(x) for x in LOp], [int(x) for x in HIp]
    SUF0 = HIp[NPAIR - 1]
    assert max(hh - ll for ll, hh in zip(LOp, HIp)) <= CHW, "mixed band exceeds et tile"

    one_f8 = np.array(1.0, dtype=F8).view(np.uint8)

    def to_pair_stat(x65):  # [4096, 65] f32 -> [128, NPAIR, 2, MPAD] fp8
        t = np.zeros((128, NPAIR, 2, MPAD), np.float32)
        t[:, :, :, 0:F_OUT + 1] = x65.reshape(NPAIR, 2, 128, F_OUT + 1).transpose(2, 0, 1, 3)
        assert np.abs(t).max() <= 240.0, "fp8 e4m3 overflow in stationary"
        return np.ascontiguousarray(t.astype(F8))

    def to_top_stat(x65):  # top-pair rows [3840:4096] -> [128, 2, 65] bf16
        return _cast_bf16(np.ascontiguousarray(
            x65[-256:].reshape(2, 128, F_OUT + 1).transpose(1, 0, 2)
        ))

    # global power-of-two scale so u_j*hp1 fits e4m3 (max 240); applied back
    # on the host during the suffix combine.  Shared across heads (SPMD).
    maxB = 0.0
    hps, us, vs = [], [], []
    for c in range(H):
        perm, ss = perms[c], s_sorted_all[c]
        hp = (h[perm].astype(np.float64) @ w[c].astype(np.float64)).astype(np.float32)
        hp1 = np.concatenate([hp, np.ones((N, 1), np.float32)], axis=1)
        u_full = np.exp(ss).astype(np.float32)
        v_full = np.exp(0.2 * ss).astype(np.float32)
        maxB = max(maxB, float(np.abs(hp1[:-256] * u_full[:-256, None]).max()))
        hps.append(hp1); us.append(u_full); vs.append(v_full)
    CB = 2 ** math.ceil(math.log2(maxB / 240.0))

    in_maps = []
    for c in range(H):
        perm, ss = perms[c], s_sorted_all[c]
        # paired blocked permuted transposed adjacency, half-major, two pairs
        # per contiguous transfer block:
        # adjm[half, pq, p, q, e, i'] = adj[perm[half*2048+i'], perm[((2pq+q)*2+e)*128+p]]
        G = adj_u8[perm][:, perm]
        blk_p = (np.ascontiguousarray(G.T).reshape(NPQ, 2, 2, 128, N) * one_f8)
        adjm_c = np.ascontiguousarray(
            blk_p.reshape(NPQ, 2, 2, 128, 2, HALFW).transpose(4, 0, 3, 1, 2, 5)
        ).view(F8)

        hp1, u_full, v_full = hps[c], us[c], vs[c]
        Bfull = hp1 * u_full[:, None] / CB
        Bfull[-256:] = 0.0  # top pair runs the bf16 path; keep fp8 in range
        s_col = ss.reshape(NJT, 128).T
        uv_c = np.stack(
            [np.exp(s_col), np.exp(0.2 * s_col)], axis=1
        ).astype(np.float32)
        wrow_c = _cast_bf16(np.exp(0.8 * ss).astype(np.float32))
        hp1_bf = _cast_bf16(np.ascontiguousarray(
            hp1.reshape(NJT, 128, F_OUT + 1).transpose(1, 0, 2)
        ))
        in_maps.append(
            {
                "adjm": adjm_c,
                "AstD": to_pair_stat(hp1 * v_full[:, None]),
                "BstD": to_pair_stat(Bfull),
                "AstT": to_top_stat(hp1 * v_full[:, None]),
                "BstT": to_top_stat(hp1 * u_full[:, None] / CB),
                "hp1D": hp1_bf,
                "uv": np.ascontiguousarray(uv_c),
                "wrowh": wrow_c,
            }
        )

    key = (tuple(LOp), tuple(HIp))
    if _CACHED_NC is None or _CACHED_KEY != key:
        _CACHED_NC = build_nc((LOp, HIp))
        _split_excess_waits(_CACHED_NC)  # HW-only fixup; CoreSim rejects the NoOps
        _CACHED_KEY = key
    res = run_bass_kernel_spmd(_CACHED_NC, in_maps, list(range(H)))
    LAST_RESULTS = res

    # host finalize: combine prefix + CB*w_i*suffix, divide by rowsum, bias,
    # output LeakyReLU(0.01), unpermute
    out = np.empty((H, N, F_OUT), dtype=np.float32)
    for c in range(H):
        P = np.asarray(res.results[c]["outP"]).astype(np.float64)  # [65, 4096]
        S = np.asarray(res.results[c]["outS"]).astype(np.float64)
        S[:, :SUF0] = 0.0
        wr = np.exp(0.8 * s_sorted_all[c]) * CB
        t = P + S * wr[None, :]
        a = (t[0:F_OUT] / t[F_OUT:F_OUT + 1]).T + bias[None, :]
        out[c, perms[c], :] = np.where(a >= 0, a, 0.01 * a)
    return out
